# revision 46
# baseline (speedup 1.0000x reference)
"""Trainium2 Bass kernel for Conv2d_NN (k-NN gather + grouped conv1d).

Shapes (hardcoded): x (32, 32, 96, 96) f32, conv_w (256, 128, 9) f32,
conv_b (256,) f32 -> out (32, 64, 96, 96) f32.

Strategy: data-parallel over batch across 8 NeuronCores (4 batches/core).
Per batch on device (tokens N=2304, features D=128 after host pixel-unshuffle):
  - scores = x^T x - 0.5*||x_j||^2 via fp16 split-precision PE matmuls:
    x = h + l (both fp16) gives dot = h.h' + h.l' + l.h' (~22-bit exact);
    the exact -0.5*nsq row is host-computed, fp16 hi+lo split, and folded in
    as a single 2-partition-contraction matmul; self-match is excluded with
    a -60000*I fp16 diagonal matmul
  - scores accumulate in two PSUM half-tiles so the ACT evacuation of half A
    overlaps the matmuls of half B
  - DVE vector.max / vector.max_index give each token's top-8 neighbors
    (self is rank 0 and needs no gather)
  - a 2-hop DMA shuffle builds the wrapped index layout for gpsimd.ap_gather,
    which gathers neighbor columns of x2p (each f32 word = duplicated bf16
    pair, so the conv can read stride-2 bf16 views)
  - conv1d == 9 accumulating bf16 128x128 matmuls per output half; ACT adds
    bias + ReLU; DMA writes (b, 256, 2304); host does the pixel-shuffle back.
"""

import sys

for _p in ("/opt/trn_rl_repo",):
    if _p not in sys.path:
        sys.path.insert(0, _p)

import numpy as np

import concourse.bass as bass
import concourse.mybir as mybir
import concourse.tile as tile
from concourse import bacc, bass_utils

# Problem constants
B, C_IN, C_OUT, H, W = 32, 32, 64, 96, 96
S = 2
K = 9
D = C_IN * S * S            # 128
D_OUT = C_OUT * S * S       # 256
N = (H // S) * (W // S)     # 2304
NCORES = 8
BPC = B // NCORES           # 4 batches per core

P = 128                     # partitions / m-tile size
NT = N // P                 # 18 m-tiles
CHUNK = 512                 # psum bank = 512 f32
CHUNKS = [(c, min(CHUNK, N - c)) for c in range(0, N, CHUNK)]  # 4x512 + 256
NEGBIG = -60000.0           # self-exclusion (fp16-safe)
CONV_GROUP = 4              # m-tiles per conv group (512 tokens)
HALF_A = 1024               # scores PSUM piece A columns (2 banks)
HALF_B = 1024               # piece B columns (2 banks)
HALF_C = N - HALF_A - HALF_B  # trailing piece C (256 cols, 1 bank)

_cache = {}


def _build_kernel(bpc=BPC, nt=NT):
    key = ("nc", bpc, nt)
    if key in _cache:
        return _cache[key], None

    nc = bacc.Bacc("TRN2", target_bir_lowering=False, debug=False)

    f32 = mybir.dt.float32
    f16 = mybir.dt.float16
    bf16 = mybir.dt.bfloat16
    u16 = mybir.dt.uint16
    i16 = mybir.dt.int16

    xh_d = nc.dram_tensor("xh", [bpc, D, N], f16, kind="ExternalInput")
    xl_d = nc.dram_tensor("xl", [bpc, D, N], f16, kind="ExternalInput")
    nsq_d = nc.dram_tensor("nsq2", [bpc, 2, N], f16, kind="ExternalInput")
    x2p_d = nc.dram_tensor("x2p", [bpc, D, N], f32, kind="ExternalInput")
    wt_d = nc.dram_tensor("wt", [D, K, 2, P], bf16, kind="ExternalInput")
    bias_d = nc.dram_tensor("bias", [P, 2], f32, kind="ExternalInput")
    ident_d = nc.dram_tensor("ident", [P, P], f16, kind="ExternalInput")
    negbig_d = nc.dram_tensor("negbig", [P, P], f16, kind="ExternalInput")
    ones2_d = nc.dram_tensor("ones2", [2, P], f16, kind="ExternalInput")
    out_d = nc.dram_tensor("out", [bpc, D_OUT, N], f32, kind="ExternalOutput")

    with tile.TileContext(nc) as tc:
        import contextlib

        with contextlib.ExitStack() as ctx:
            const_pool = ctx.enter_context(tc.tile_pool(name="consts", bufs=1))
            xh_pool = ctx.enter_context(tc.tile_pool(name="xh", bufs=4))
            xl_pool = ctx.enter_context(tc.tile_pool(name="xl", bufs=4))
            nsq_pool = ctx.enter_context(tc.tile_pool(name="nsq", bufs=4))
            x2p_pool = ctx.enter_context(tc.tile_pool(name="x2p", bufs=4))
            scores_pool = ctx.enter_context(tc.tile_pool(name="scores", bufs=4))
            mx_pool = ctx.enter_context(tc.tile_pool(name="mx", bufs=8))
            widx_pool = ctx.enter_context(tc.tile_pool(name="widx", bufs=8))
            g_pool = ctx.enter_context(tc.tile_pool(name="g", bufs=3))
            outs_pool = ctx.enter_context(tc.tile_pool(name="outs", bufs=4))
            psum_pool = ctx.enter_context(
                tc.tile_pool(name="psum", bufs=1, space="PSUM")
            )
            psum_pool2 = ctx.enter_context(
                tc.tile_pool(name="psum2", bufs=1, space="PSUM")
            )
            psum_pool3 = ctx.enter_context(
                tc.tile_pool(name="psum3", bufs=1, space="PSUM")
            )
            psum_conv_pool = ctx.enter_context(
                tc.tile_pool(name="psumc", bufs=3, space="PSUM")
            )
            dram_pool = ctx.enter_context(
                tc.tile_pool(name="stage", bufs=8, space="DRAM")
            )

            # constants, loaded once
            wt_s = const_pool.tile([D, K * 2 * P], bf16, tag="wt")
            nc.sync.dma_start(wt_s[:], wt_d.ap().rearrange("d k h c -> d (k h c)"))
            wt_v = wt_s[:].rearrange("d (k h c) -> d k h c", k=K, h=2, c=P)
            bias_s = const_pool.tile([P, 2], f32, tag="bias")
            nc.sync.dma_start(bias_s[:], bias_d.ap())
            ident_s = const_pool.tile([P, P], f16, tag="ident")
            nc.sync.dma_start(ident_s[:], ident_d.ap())
            negbig_s = const_pool.tile([P, P], f16, tag="negbig")
            nc.sync.dma_start(negbig_s[:], negbig_d.ap())
            ones2_s = const_pool.tile([2, P], f16, tag="ones2")
            nc.sync.dma_start(ones2_s[:], ones2_d.ap())

            def batch_prep(b):
                """Input loads for batch b (no device pre-compute needed).

                xh/xl arrive in two half-loads ordered so the first tile's
                half-A matmuls only wait on the leading pieces.
                """
                xh = xh_pool.tile([D, N], f16)
                xl = xl_pool.tile([D, N], f16)
                nsq2 = nsq_pool.tile([2, N], f16)
                nc.sync.dma_start(xh[:, :HALF_A], xh_d.ap()[b][:, :HALF_A])
                nc.sync.dma_start(xl[:, :HALF_A], xl_d.ap()[b][:, :HALF_A])
                nc.sync.dma_start(nsq2[:], nsq_d.ap()[b])
                nc.sync.dma_start(xh[:, HALF_A:], xh_d.ap()[b][:, HALF_A:])
                nc.sync.dma_start(xl[:, HALF_A:], xl_d.ap()[b][:, HALF_A:])
                x2p = x2p_pool.tile([D, N], f32)
                nc.sync.dma_start(x2p[:], x2p_d.ap()[b])
                return xh, xl, nsq2, x2p

            def emit_conv(b, x2p, g_group, group_start, group_tiles):
                """conv1d over a completed gather group + ReLU + out DMA.

                All operands are bf16 (stride-2 views of the bf16-pair
                packing); PSUM accumulates f32.
                """
                gtok = group_tiles * P
                gv = g_group[:, : group_tiles * 1024].bitcast(bf16).rearrange(
                    "d (mt u k r two) -> d mt u k r two",
                    mt=group_tiles, u=8, k=8, r=16, two=2,
                )
                x2pb = x2p[:].bitcast(bf16).rearrange("d (n two) -> d n two", two=2)
                for h in range(2):
                    cp = psum_conv_pool.tile([P, CHUNK], f32, tag="pconv")
                    # k = 0: self columns, no gather needed
                    nc.tensor.matmul(
                        cp[:, :gtok],
                        lhsT=wt_v[:, 0, h, :],
                        rhs=x2pb[:, group_start : group_start + gtok, 0],
                        start=True, stop=False,
                    )
                    for k in range(1, K):
                        nc.tensor.matmul(
                            cp[:, :gtok],
                            lhsT=wt_v[:, k, h, :],
                            rhs=gv[:, :, :, k - 1, :, 0],
                            start=False, stop=(k == K - 1),
                        )
                    o_s = outs_pool.tile([P, CHUNK], f32)
                    nc.scalar.activation(
                        o_s[:, :gtok], cp[:, :gtok],
                        mybir.ActivationFunctionType.Relu,
                        bias=bias_s[:, h : h + 1],
                    )
                    nc.sync.dma_start(
                        out_d.ap()[b, h * P : (h + 1) * P,
                                   group_start : group_start + gtok],
                        o_s[:, :gtok],
                    )

            prepped = batch_prep(0)
            next_prepped = None
            pending_conv = None

            for b in range(bpc):
                xh, xl, nsq2, x2p = prepped

                # conv group state
                gstate = {"g_group": None, "tiles": 0, "start": 0, "len": 0}
                pending_scan = None

                def finish_scan(mt, scores, mx8, b=b, x2p=x2p, gstate=gstate):
                    """max_index + index shuffle + gather for a scanned tile;
                    deferred one tile so max_index never waits on max8's
                    write-ack."""
                    nonlocal pending_conv
                    m0 = mt * P
                    midx = mx_pool.tile([P, 8], u16, tag="midx")
                    nc.vector.max_index(midx[:], mx8[:], scores[:])

                    # ---- 2-hop DMA shuffle to wrapped gather-index layout:
                    # staging[r*64 + u*8 + (k-1)] = midx[u*16+r, k-1]
                    stage_t = dram_pool.tile([1, 1024], u16)
                    st_dst = stage_t[:].rearrange(
                        "a (r u k) -> a u r k", r=16, u=8, k=8
                    ).squeeze(0)
                    nc.sync.dma_start(st_dst, midx[:])
                    # widx[16c+r, c2] = staging[r*64 + c2] (c replicated, 0-step)
                    widx = widx_pool.tile([P, 64], i16)
                    st_src = (
                        stage_t[:]
                        .rearrange("a (r c2) -> a r c2", r=16, c2=64)
                        .unsqueeze(1)
                        .broadcast_to([1, 8, 16, 64])
                        .bitcast(i16)
                        .squeeze(0)
                    )
                    nc.sync.dma_start(widx[:], st_src)

                    # ---- gather neighbors k=1..8 on gpsimd ----
                    if gstate["tiles"] == 0:
                        g_new = g_pool.tile([D, CONV_GROUP * 1024], f32, tag="g")
                        gstate["g_group"] = g_new
                        gstate["start"] = m0
                        gstate["len"] = min(CONV_GROUP, nt - mt)
                    gt_ = gstate["tiles"]
                    nc.gpsimd.ap_gather(
                        gstate["g_group"][:, gt_ * 1024 : (gt_ + 1) * 1024],
                        x2p[:],
                        widx[:],
                        channels=P,
                        num_elems=N,
                        d=1,
                        num_idxs=1024,
                    )
                    gstate["tiles"] += 1
                    if gstate["tiles"] == gstate["len"]:
                        pending_conv = (
                            b, x2p, gstate["g_group"], gstate["start"],
                            gstate["tiles"],
                        )
                        gstate["tiles"] = 0

                for mt in range(nt):
                    m0 = mt * P
                    # ---- scores matmuls into two PSUM half-tiles; evac of
                    # half A overlaps the matmuls of half B ----
                    scores = scores_pool.tile([P, N], f32)
                    scpA = psum_pool.tile([P, HALF_A], f32, tag="scoresA")
                    scpB = psum_pool2.tile([P, HALF_B], f32, tag="scoresB")
                    scpC = psum_pool3.tile([P, HALF_C], f32, tag="scoresC")
                    halves = [(scpA, 0, HALF_A), (scpB, HALF_A, HALF_B),
                              (scpC, HALF_A + HALF_B, HALF_C)]
                    for scp, h0, hw_ in halves:
                        hchunks = [
                            (c0, w) for c0, w in CHUNKS if h0 <= c0 < h0 + hw_
                        ]
                        # dot = h.h' + h.l' + l.h'  (fp16 split precision)
                        for c0, w in hchunks:
                            nc.tensor.matmul(
                                scp[:, c0 - h0 : c0 - h0 + w],
                                lhsT=xh[:, m0 : m0 + P],
                                rhs=xh[:, c0 : c0 + w],
                                start=True, stop=False,
                            )
                            nc.tensor.matmul(
                                scp[:, c0 - h0 : c0 - h0 + w],
                                lhsT=xh[:, m0 : m0 + P],
                                rhs=xl[:, c0 : c0 + w],
                                start=False, stop=False,
                            )
                            nc.tensor.matmul(
                                scp[:, c0 - h0 : c0 - h0 + w],
                                lhsT=xl[:, m0 : m0 + P],
                                rhs=xh[:, c0 : c0 + w],
                                start=False, stop=False,
                            )
                        if h0 <= m0 < h0 + hw_:
                            # self-exclusion: scores[p, m0+p] += NEGBIG
                            nc.tensor.matmul(
                                scp[:, m0 - h0 : m0 - h0 + P],
                                lhsT=negbig_s[:],
                                rhs=ident_s[:],
                                start=False, stop=False,
                            )
                        # exact -0.5*nsq row: hi+lo fp16 rows in a single
                        # 2-partition-contraction matmul; closes the groups
                        for c0, w in hchunks:
                            nc.tensor.matmul(
                                scp[:, c0 - h0 : c0 - h0 + w],
                                lhsT=ones2_s[:],
                                rhs=nsq2[:, c0 : c0 + w],
                                start=False, stop=True,
                            )
                        nc.scalar.copy(scores[:, h0 : h0 + hw_], scp[:])

                    # deferred conv for the previously completed group: PE
                    # runs it after this tile's scores so evac/DVE aren't
                    # stalled behind it
                    if pending_conv is not None:
                        emit_conv(*pending_conv)
                        pending_conv = None

                    # prefetch next batch's inputs near batch end
                    if mt == nt - 4 and b + 1 < bpc:
                        next_prepped = batch_prep(b + 1)

                    # ---- top-8 on DVE ----
                    mx8 = mx_pool.tile([P, 8], f32, tag="mx8")
                    nc.vector.max(out=mx8[:], in_=scores[:])
                    finish_scan(mt, scores, mx8)

                prepped = next_prepped
                next_prepped = None

            # final group's conv
            if pending_conv is not None:
                emit_conv(*pending_conv)
                pending_conv = None

    nc.compile()
    _cache[key] = nc
    return nc, None


def _host_inputs(x, conv_w, conv_b):
    """Shared per-core constant inputs + per-core per-batch tensors."""
    import ml_dtypes

    x = np.ascontiguousarray(x, dtype=np.float32)
    b = x.shape[0]
    x1 = (
        x.reshape(b, C_IN, H // S, S, W // S, S)
        .transpose(0, 1, 3, 5, 2, 4)
        .reshape(b, D, N)
    )
    # fp16 split: x = xh + xl, each fp16 (~22-bit combined)
    xh = x1.astype(np.float16)
    xl = (x1 - xh.astype(np.float32)).astype(np.float16)
    # exact -0.5*||x_j||^2 row, fp16 hi+lo split
    nsqrow = (-0.5 * np.einsum(
        "bdn,bdn->bn", x1.astype(np.float64), x1.astype(np.float64)
    )).astype(np.float32)
    nh = nsqrow.astype(np.float16)
    nl = (nsqrow - nh.astype(np.float32)).astype(np.float16)
    nsq2 = np.ascontiguousarray(np.stack([nh, nl], axis=1))  # [b, 2, N]
    # bf16-pair packing: each f32 word of x2p = (bf16(x) << 16) | bf16(x)
    xb = x1.astype(ml_dtypes.bfloat16).view(np.uint16).astype(np.uint32)
    x2p = ((xb << 16) | xb).view(np.float32)
    wt = np.ascontiguousarray(
        conv_w.reshape(2, P, D, K).transpose(2, 3, 0, 1).astype(ml_dtypes.bfloat16)
    )  # [D, K, 2, P]; conv_w is (256,128,9) -> (2,128half) x d x k
    bias = np.ascontiguousarray(
        conv_b.reshape(2, P).transpose(1, 0), dtype=np.float32
    )  # [P, 2]
    ident = np.eye(P, dtype=np.float16)
    negbig = (NEGBIG * np.eye(P)).astype(np.float16)
    ones2 = np.ones((2, P), dtype=np.float16)
    per_batch = dict(xh=xh, xl=xl, nsq2=nsq2, x2p=x2p)
    consts = dict(wt=wt, bias=bias, ident=ident, negbig=negbig, ones2=ones2)
    return per_batch, consts


def kernel(x, conv_w, conv_b):
    nc, _ = _build_kernel()
    per_batch, consts = _host_inputs(x, conv_w, conv_b)
    in_maps = []
    for c in range(NCORES):
        m = dict(consts)
        for k, v in per_batch.items():
            m[k] = np.ascontiguousarray(v[c * BPC : (c + 1) * BPC])
        in_maps.append(m)
    res = bass_utils.run_bass_kernel_spmd(nc, in_maps, core_ids=list(range(NCORES)))
    outs = np.concatenate([r["out"] for r in res.results], axis=0)  # [B, 256, N]
    # pixel shuffle back: channel dim = (co, sy, sx); token = (h, w)
    o = outs.reshape(B, C_OUT, S, S, H // S, W // S)
    o = o.transpose(0, 1, 4, 2, 5, 3).reshape(B, C_OUT, H, W)
    return np.ascontiguousarray(o, dtype=np.float32)
